# revision 12
# baseline (speedup 1.0000x reference)
"""Trainium2 Bass kernel for NeuralTensorDiagLayer (B=8192, D=K=2048).

Math: reference computes
    ff = concat([e1, e2], -1) @ V                      # (B, K)
    u  = ((e1*e2) @ W.T) / K                           # (B, K)
    z[p, q] = u[(p*D + q) % B, q]
    out = tanh(z + ff + b)

With D=2048, B=8192: (p*D + q) % B == 2048*(p % 4) + q, so z has only 4
distinct rows: zrow[r, q] = u[2048*r + q, q] — the diagonals of the four
2048x2048 blocks of u. Hence the big u GEMM is unnecessary:
    zrow[r, q] = (1/K) * sum_j h[2048*r + q, j] * W[q, j],  h = e1*e2
which is an elementwise multiply + reduction only.

Distribution (8 cores, data-parallel over batch):
  core i owns rows [1024*i, 1024*(i+1)).  Those rows are exactly
  block r = i//2, q in [1024*(i%2), 1024*(i%2)+1024) of zrow, i.e.
  zflat[1024*i : 1024*(i+1)] where zflat = zrow.flatten().  Each core
  computes its slice from its own e1/e2 shard + a 1024-row slice of W,
  then a tiny AllGather (4KB/core) replicates zflat to all cores.

Per-core device work:
  - main GEMM (transposed layout, float32r for full-rate PE):
    out_T[q, p] = sum_k V[k, q] * XT[k, p], XT = [e1s.T; e2s.T] resident
    in SBUF; lhsT = 128-col slices of streamed V tiles.
  - z slice: t = (e1s.T * e2s.T) * (W_slice.T / K) in bf16, reduced over
    the j-partition dim with a ones-vector matmul (z is ~1e-3 magnitude
    vs ff ~1, so bf16 error there is negligible).
  - epilogue: out_T = tanh(ff_T + zrow[p%4, q] + b[q]); the z term is a
    period-4 broadcast along the free dim.
Scheduling: the front-load loop interleaves XT/W DMAs (arrival-paced),
the z elementwise muls + PE reduction, and group 0's GEMM matmuls, so
the PE is busy from the first microseconds.  Output columns are
processed in groups of 3 chunks (6 PSUM banks) — 2 banks stay dedicated
to the z reduction — with the final group of 4 reusing the z banks.
Group 0's epilogue is deferred through SBUF copies so its banks free
while the z AllGather is in flight.  Host does sharding/layout only.
"""

import os
import ml_dtypes
import numpy as np

B, D, K = 8192, 2048, 2048
NCORES = 8
M = B // NCORES      # 1024 batch rows per core
KC = 2 * D           # 4096 contraction dim
P = 128
KT = KC // P         # 32 contraction chunks
NT = K // P          # 16 output-column chunks
JT = D // P          # 16 j-chunks for the z reduction
MBS = 512            # matmul moving free dim
NMB = M // MBS       # 2 m-blocks

# column-chunk groups: 6-bank groups while z owns 2 banks, 8-bank final
GROUPS = [[0, 1, 2], [3, 4, 5], [6, 7, 8], [9, 10, 11], [12, 13, 14, 15]]

# contraction order matched to the front-load DMA arrival order
KORDER = []
for _j in range(JT):
    KORDER.extend([_j, JT + _j])

_cache = {}
LAST_RESULT = None


def _build():
    import concourse.bass as bass
    import concourse.mybir as mybir
    import concourse.tile as tile
    from concourse import bacc

    f32 = mybir.dt.float32
    f32r = mybir.dt.float32r
    bf16 = mybir.dt.bfloat16
    Act = mybir.ActivationFunctionType

    nc = bacc.Bacc(
        "TRN2", target_bir_lowering=False, debug=False, num_devices=NCORES
    )

    xt = nc.dram_tensor("xt", [KC, M], f32r, kind="ExternalInput").ap()
    v = nc.dram_tensor("v", [KC, K], f32r, kind="ExternalInput").ap()
    wt = nc.dram_tensor("wt", [D, M], bf16, kind="ExternalInput").ap()
    bvec = nc.dram_tensor("bvec", [K], f32, kind="ExternalInput").ap()
    out_t = nc.dram_tensor("out_t", [K, M], f32, kind="ExternalOutput").ap()

    with tile.TileContext(nc) as tc:
        with (
            tc.tile_pool(name="xtp", bufs=1) as xtp,
            tc.tile_pool(name="vp", bufs=12) as vp,
            tc.tile_pool(name="wtp", bufs=2) as wtp,
            tc.tile_pool(name="htp", bufs=2) as htp,
            tc.tile_pool(name="ttp", bufs=3) as ttp,
            tc.tile_pool(name="outp", bufs=7) as outp,
            tc.tile_pool(name="constp", bufs=1) as constp,
            tc.tile_pool(name="psg", bufs=1, space="PSUM") as psg,
            tc.tile_pool(name="dramp", bufs=1, space="DRAM") as dramp,
        ):
            ones = constp.tile([P, 1], bf16, name="ones", tag="ones")
            nc.vector.memset(ones[:], 1.0)
            b_all = constp.tile([P, NT], f32, name="b_all", tag="b_all")
            nc.sync.dma_start(b_all[:], bvec.rearrange("(n q) -> q n", q=P))
            zq_all = constp.tile([P, NT, 4], f32, name="zq_all", tag="zq_all")

            def group_psums(g):
                cols = GROUPS[g]
                return [
                    psg.tile([P, MBS], f32, name=f"ps{s}",
                             tag=f"ps{s}" if s < 6 else f"pz{s - 6}")
                    for s in range(len(cols) * NMB)
                ]

            def group_matmuls(g, pss, kk, idx, vts):
                cols = GROUPS[g]
                c0 = cols[0]
                vt = vts.get(kk)
                if vt is None:
                    vt = vp.tile([P, len(cols) * P], f32r, name="vt",
                                 tag=f"vt{len(cols)}")
                    nc.sync.dma_start(
                        vt[:],
                        v[kk * P : (kk + 1) * P,
                          c0 * P : (cols[-1] + 1) * P],
                    )
                    vts[kk] = vt
                for ci in range(len(cols)):
                    for mb in range(NMB):
                        nc.tensor.matmul(
                            pss[ci * NMB + mb][:],
                            vt[:, ci * P : (ci + 1) * P],
                            xts[kk][:, mb * MBS : (mb + 1) * MBS],
                            start=(idx == 0), stop=(idx == KT - 1),
                        )

            pending = []

            def copy_chunk(n, src0, src1):
                """ACT-copy the two [P, MBS] psum halves of chunk n into an
                SBUF tile — frees the banks without waiting for z."""
                osb = outp.tile([P, M], f32, name="osb")
                nc.vector.tensor_copy(osb[:, 0:MBS], src0[:])
                nc.vector.tensor_copy(osb[:, MBS:M], src1[:])
                pending.append((n, osb))

            def finish_pending():
                """z-add (in place), tanh+bias, store — on queues whose
                later work is also z-dependent (vector/scalar/gpsimd)."""
                for n, osb in pending:
                    zq_b = zq_all[:, n : n + 1, :].broadcast_to(
                        [P, MBS // 4, 4])
                    for mb in range(NMB):
                        h = osb[:, mb * MBS : (mb + 1) * MBS].rearrange(
                            "p (a r) -> p a r", r=4)
                        nc.vector.tensor_add(h, h, zq_b)
                    nc.scalar.activation(
                        osb[:], osb[:], Act.Tanh, bias=b_all[:, n : n + 1]
                    )
                    nc.gpsimd.dma_start(out_t[n * P : (n + 1) * P, :],
                                        osb[:])
                pending.clear()

            # z-reduction accumulators (dedicated banks until the final
            # GEMM group, whose last 2 psum tiles reuse these slots)
            pz0 = psg.tile([1, MBS], f32, name="pz0", tag="pz0")
            pz1 = psg.tile([1, MBS], f32, name="pz1", tag="pz1")

            # ---- front-load: XT/W DMAs + z path + group-0 GEMM ----
            xts = [None] * KT
            pss0 = group_psums(0)
            vts0 = {}
            for j in range(JT):
                for kk in (j, JT + j):
                    t = xtp.tile([P, M], f32r, name=f"xt{kk}", tag=f"xt{kk}")
                    nc.sync.dma_start(t[:], xt[kk * P : (kk + 1) * P, :])
                    xts[kk] = t
                wtj = wtp.tile([P, M], bf16, name="wtj")
                nc.sync.dma_start(wtj[:], wt[j * P : (j + 1) * P, :])
                htj = htp.tile([P, M], f32, name="htj")
                nc.vector.tensor_mul(
                    htj[:], xts[j][:].bitcast(f32), xts[JT + j][:].bitcast(f32)
                )
                ttj = ttp.tile([P, M], bf16, name="ttj")
                nc.vector.tensor_mul(ttj[:], htj[:], wtj[:])
                group_matmuls(0, pss0, j, 2 * j, vts0)
                group_matmuls(0, pss0, JT + j, 2 * j + 1, vts0)
                nc.tensor.matmul(pz0[:], ones[:], ttj[:, 0:MBS],
                                 start=(j == 0), stop=(j == JT - 1))
                nc.tensor.matmul(pz1[:], ones[:], ttj[:, MBS:M],
                                 start=(j == 0), stop=(j == JT - 1))

            # group-0 psum -> SBUF copies (free the banks immediately;
            # the z-dependent finish happens after the AllGather)
            for ci, n in enumerate(GROUPS[0]):
                copy_chunk(n, pss0[ci * NMB], pss0[ci * NMB + 1])

            # ---- z slice -> AllGather -> zq_all ----
            zsl = constp.tile([1, M], f32, name="zsl", tag="zsl")
            nc.scalar.activation(zsl[:, 0:MBS], pz0[:], Act.Copy)
            nc.scalar.activation(zsl[:, MBS:M], pz1[:], Act.Copy)
            zin = dramp.tile([M], f32, name="zin", tag="zin")
            zout = dramp.tile([B], f32, name="zout", tag="zout",
                              addr_space="Shared")
            nc.scalar.dma_start(zin[:], zsl[:])
            nc.gpsimd.collective_compute(
                "AllGather",
                mybir.AluOpType.bypass,
                replica_groups=[list(range(NCORES))],
                ins=[zin[:].opt()],
                outs=[zout[:].opt()],
            )
            # zq_all[qq, n, r] = zflat[2048*r + 128*n + qq] = zrow[r, 128n+qq]
            for r in range(4):
                nc.gpsimd.dma_start(
                    zq_all[:, :, r],
                    zout[r * D : (r + 1) * D].rearrange("(n q) -> q n", q=P),
                )



            # ---- middle groups: GEMM, bank-freeing copies, deferred
            # finishing of the previous group's chunks ----
            for g in range(1, len(GROUPS) - 1):
                pss = group_psums(g)
                vts = {}
                prev = list(pending)
                pending.clear()
                for idx, kk in enumerate(KORDER):
                    group_matmuls(g, pss, kk, idx, vts)
                for ci, n in enumerate(GROUPS[g]):
                    copy_chunk(n, pss[ci * NMB], pss[ci * NMB + 1])
                just = list(pending)
                pending.clear()
                pending.extend(prev)
                finish_pending()
                pending.extend(just)

            # ---- final group: two 2-chunk pairs on disjoint banks so
            # each pair's epilogue overlaps the next pair's GEMM ----
            PAIR_TAGS = [["ps0", "ps1", "ps2", "ps3"],
                         ["ps4", "ps5", "pz0", "pz1"]]
            for pi, pair in enumerate(([12, 13], [14, 15])):
                pps = [
                    psg.tile([P, MBS], f32, name=f"pp{pi}_{s}",
                             tag=PAIR_TAGS[pi][s])
                    for s in range(4)
                ]
                for idx, kk in enumerate(KORDER):
                    vt = vp.tile([P, 2 * P], f32r, name="vt2", tag="vt2",
                                  bufs=6)
                    nc.sync.dma_start(
                        vt[:],
                        v[kk * P : (kk + 1) * P,
                          pair[0] * P : (pair[1] + 1) * P],
                    )
                    for ci in range(2):
                        for mb in range(NMB):
                            nc.tensor.matmul(
                                pps[ci * NMB + mb][:],
                                vt[:, ci * P : (ci + 1) * P],
                                xts[kk][:, mb * MBS : (mb + 1) * MBS],
                                start=(idx == 0), stop=(idx == KT - 1),
                            )
                for ci, n in enumerate(pair):
                    copy_chunk(n, pps[ci * NMB], pps[ci * NMB + 1])
                finish_pending()

    nc.compile()
    return nc


def _get_nc():
    nc = _cache.get("nc")
    if nc is None:
        nc = _build()
        _cache["nc"] = nc
    return nc


def kernel(e1, e2, W, V, b):
    from concourse.bass_utils import run_bass_kernel_spmd

    nc = _get_nc()

    e1 = np.ascontiguousarray(np.asarray(e1, dtype=np.float32))
    e2 = np.ascontiguousarray(np.asarray(e2, dtype=np.float32))
    W = np.ascontiguousarray(np.asarray(W, dtype=np.float32))
    V = np.ascontiguousarray(np.asarray(V, dtype=np.float32))
    b = np.ascontiguousarray(np.asarray(b, dtype=np.float32))

    in_maps = []
    for i in range(NCORES):
        sl = slice(i * M, (i + 1) * M)
        xt_i = np.ascontiguousarray(
            np.concatenate([e1[sl].T, e2[sl].T], axis=0)
        )
        qlo = (i % 2) * M
        # 1/K scale folded into W (power of two — exact in fp32);
        # bf16 is plenty for the z term (|z| ~ 1e-3 vs |ff| ~ 1)
        wt_i = np.ascontiguousarray(
            (W[qlo : qlo + M].T * np.float32(1.0 / K)).astype(
                ml_dtypes.bfloat16
            )
        )
        in_maps.append({"xt": xt_i, "v": V, "wt": wt_i, "bvec": b})

    res = run_bass_kernel_spmd(nc, in_maps, list(range(NCORES)))
    global LAST_RESULT
    LAST_RESULT = res

    out = np.empty((B, K), dtype=np.float32)
    for i in range(NCORES):
        out[i * M : (i + 1) * M, :] = res.results[i]["out_t"].T
    return out


# revision 14
# speedup vs baseline: 1.0436x; 1.0436x over previous
"""Trainium2 Bass kernel for NeuralTensorDiagLayer (B=8192, D=K=2048).

Math: reference computes
    ff = concat([e1, e2], -1) @ V                      # (B, K)
    u  = ((e1*e2) @ W.T) / K                           # (B, K)
    z[p, q] = u[(p*D + q) % B, q]
    out = tanh(z + ff + b)

With D=2048, B=8192: (p*D + q) % B == 2048*(p % 4) + q, so z has only 4
distinct rows: zrow[r, q] = u[2048*r + q, q] — the diagonals of the four
2048x2048 blocks of u. Hence the big u GEMM is unnecessary:
    zrow[r, q] = (1/K) * sum_j h[2048*r + q, j] * W[q, j],  h = e1*e2
which is an elementwise multiply + reduction only.

Distribution (8 cores, data-parallel over batch):
  core i owns rows [1024*i, 1024*(i+1)).  Those rows are exactly
  block r = i//2, q in [1024*(i%2), 1024*(i%2)+1024) of zrow, i.e.
  zflat[1024*i : 1024*(i+1)] where zflat = zrow.flatten().  Each core
  computes its slice from its own e1/e2 shard + a 1024-row slice of W,
  then a tiny AllGather (4KB/core) replicates zflat to all cores.

Per-core device work:
  - main GEMM (transposed layout, float32r for full-rate PE):
    out_T[q, p] = sum_k V[k, q] * XT[k, p], XT = [e1s.T; e2s.T] resident
    in SBUF; lhsT = 128-col slices of streamed V tiles.
  - z slice: t = (e1s.T * e2s.T) * (W_slice.T / K) in bf16, reduced over
    the j-partition dim with a ones-vector matmul (z is ~1e-3 magnitude
    vs ff ~1, so bf16 error there is negligible).
  - epilogue: out_T = tanh(ff_T + zrow[p%4, q] + b[q]); the z term is a
    period-4 broadcast along the free dim.
Scheduling: the front-load loop interleaves XT/W DMAs (arrival-paced),
the z elementwise muls + PE reduction, and group 0's GEMM matmuls, so
the PE is busy from the first microseconds.  Output columns are
processed in groups of 3 chunks (6 PSUM banks) — 2 banks stay dedicated
to the z reduction — with the final group of 4 reusing the z banks.
Group 0's epilogue is deferred through SBUF copies so its banks free
while the z AllGather is in flight.  Host does sharding/layout only.
"""

import os
import ml_dtypes
import numpy as np

B, D, K = 8192, 2048, 2048
NCORES = 8
M = B // NCORES      # 1024 batch rows per core
KC = 2 * D           # 4096 contraction dim
P = 128
KT = KC // P         # 32 contraction chunks
NT = K // P          # 16 output-column chunks
JT = D // P          # 16 j-chunks for the z reduction
MBS = 512            # matmul moving free dim
NMB = M // MBS       # 2 m-blocks

# column-chunk groups: 6-bank groups while z owns 2 banks, 8-bank final
GROUPS = [[0, 1, 2], [3, 4, 5], [6, 7, 8], [9, 10, 11], [12, 13, 14, 15]]

# contraction order matched to the front-load DMA arrival order
KORDER = []
for _j in range(JT):
    KORDER.extend([_j, JT + _j])

_cache = {}
LAST_RESULT = None


def _build():
    import concourse.bass as bass
    import concourse.mybir as mybir
    import concourse.tile as tile
    from concourse import bacc
    from concourse.tile import add_dep_helper

    f32 = mybir.dt.float32
    f32r = mybir.dt.float32r
    bf16 = mybir.dt.bfloat16
    Act = mybir.ActivationFunctionType

    nc = bacc.Bacc(
        "TRN2", target_bir_lowering=False, debug=False, num_devices=NCORES
    )

    xt = nc.dram_tensor("xt", [KC, M], f32r, kind="ExternalInput").ap()
    v = nc.dram_tensor("v", [KC, K], f32r, kind="ExternalInput").ap()
    wt = nc.dram_tensor("wt", [D, M], bf16, kind="ExternalInput").ap()
    bvec = nc.dram_tensor("bvec", [K], f32, kind="ExternalInput").ap()
    out_t = nc.dram_tensor("out_t", [K, M], f32, kind="ExternalOutput").ap()

    with tile.TileContext(nc) as tc:
        with (
            tc.tile_pool(name="xtp", bufs=1) as xtp,
            tc.tile_pool(name="vp", bufs=12) as vp,
            tc.tile_pool(name="wtp", bufs=2) as wtp,
            tc.tile_pool(name="htp", bufs=2) as htp,
            tc.tile_pool(name="ttp", bufs=3) as ttp,
            tc.tile_pool(name="outp", bufs=7) as outp,
            tc.tile_pool(name="constp", bufs=1) as constp,
            tc.tile_pool(name="psg", bufs=1, space="PSUM") as psg,
            tc.tile_pool(name="dramp", bufs=1, space="DRAM") as dramp,
        ):
            ones = constp.tile([P, 1], bf16, name="ones", tag="ones")
            nc.vector.memset(ones[:], 1.0)
            b_all = constp.tile([P, NT], f32, name="b_all", tag="b_all")
            nc.sync.dma_start(b_all[:], bvec.rearrange("(n q) -> q n", q=P))
            zq_all = constp.tile([P, NT, 4], f32, name="zq_all", tag="zq_all")

            def group_psums(g):
                cols = GROUPS[g]
                return [
                    psg.tile([P, MBS], f32, name=f"ps{s}",
                             tag=f"ps{s}" if s < 6 else f"pz{s - 6}")
                    for s in range(len(cols) * NMB)
                ]

            def group_matmuls(g, pss, kk, idx, vts):
                cols = GROUPS[g]
                c0 = cols[0]
                vt = vts.get(kk)
                if vt is None:
                    vt = vp.tile([P, len(cols) * P], f32r, name="vt",
                                 tag=f"vt{len(cols)}")
                    nc.sync.dma_start(
                        vt[:],
                        v[kk * P : (kk + 1) * P,
                          c0 * P : (cols[-1] + 1) * P],
                    )
                    vts[kk] = vt
                for ci in range(len(cols)):
                    for mb in range(NMB):
                        nc.tensor.matmul(
                            pss[ci * NMB + mb][:],
                            vt[:, ci * P : (ci + 1) * P],
                            xts[kk][:, mb * MBS : (mb + 1) * MBS],
                            start=(idx == 0), stop=(idx == KT - 1),
                        )

            pending = []
            last_copy = [None]

            def copy_chunk(n, src0, src1):
                """DVE-copy the two [P, MBS] psum halves of chunk n into an
                SBUF tile — frees the banks without waiting for z."""
                osb = outp.tile([P, M], f32, name="osb")
                nc.vector.tensor_copy(osb[:, 0:MBS], src0[:])
                last_copy[0] = nc.vector.tensor_copy(osb[:, MBS:M], src1[:])
                pending.append((n, osb))

            def finish_pending():
                """z-add (in place), tanh+bias, store — on queues whose
                later work is also z-dependent (vector/scalar/gpsimd)."""
                for n, osb in pending:
                    zq_b = zq_all[:, n : n + 1, :].broadcast_to(
                        [P, MBS // 4, 4])
                    for mb in range(NMB):
                        h = osb[:, mb * MBS : (mb + 1) * MBS].rearrange(
                            "p (a r) -> p a r", r=4)
                        add = nc.vector.tensor_add(h, h, zq_b)
                        if last_copy[0] is not None:
                            # ordering-only dep: keep the z-blocked adds
                            # behind the latest bank-freeing copy on DVE
                            add_dep_helper(
                                add.ins, last_copy[0].ins, sync=False,
                                reason="z-adds after bank-freeing copies",
                            )
                    nc.scalar.activation(
                        osb[:], osb[:], Act.Tanh, bias=b_all[:, n : n + 1]
                    )
                    nc.gpsimd.dma_start(out_t[n * P : (n + 1) * P, :],
                                        osb[:])
                pending.clear()

            # z-reduction accumulators (dedicated banks until the final
            # GEMM group, whose last 2 psum tiles reuse these slots)
            pz0 = psg.tile([1, MBS], f32, name="pz0", tag="pz0")
            pz1 = psg.tile([1, MBS], f32, name="pz1", tag="pz1")

            # ---- front-load: XT/W DMAs + z path + group-0 GEMM ----
            xts = [None] * KT
            pss0 = group_psums(0)
            vts0 = {}
            for j in range(JT):
                for kk in (j, JT + j):
                    t = xtp.tile([P, M], f32r, name=f"xt{kk}", tag=f"xt{kk}")
                    nc.sync.dma_start(t[:], xt[kk * P : (kk + 1) * P, :])
                    xts[kk] = t
                wtj = wtp.tile([P, M], bf16, name="wtj")
                nc.sync.dma_start(wtj[:], wt[j * P : (j + 1) * P, :])
                htj = htp.tile([P, M], f32, name="htj")
                nc.vector.tensor_mul(
                    htj[:], xts[j][:].bitcast(f32), xts[JT + j][:].bitcast(f32)
                )
                ttj = ttp.tile([P, M], bf16, name="ttj")
                nc.vector.tensor_mul(ttj[:], htj[:], wtj[:])
                group_matmuls(0, pss0, j, 2 * j, vts0)
                group_matmuls(0, pss0, JT + j, 2 * j + 1, vts0)
                nc.tensor.matmul(pz0[:], ones[:], ttj[:, 0:MBS],
                                 start=(j == 0), stop=(j == JT - 1))
                nc.tensor.matmul(pz1[:], ones[:], ttj[:, MBS:M],
                                 start=(j == 0), stop=(j == JT - 1))

            # group-0 psum -> SBUF copies (free the banks immediately;
            # the z-dependent finish happens after the AllGather)
            for ci, n in enumerate(GROUPS[0]):
                copy_chunk(n, pss0[ci * NMB], pss0[ci * NMB + 1])

            # ---- z slice -> AllGather -> zq_all ----
            zsl = constp.tile([1, M], f32, name="zsl", tag="zsl")
            nc.scalar.activation(zsl[:, 0:MBS], pz0[:], Act.Copy)
            nc.scalar.activation(zsl[:, MBS:M], pz1[:], Act.Copy)
            zin = dramp.tile([M], f32, name="zin", tag="zin")
            zout = dramp.tile([B], f32, name="zout", tag="zout",
                              addr_space="Shared")
            nc.scalar.dma_start(zin[:], zsl[:])
            nc.gpsimd.collective_compute(
                "AllGather",
                mybir.AluOpType.bypass,
                replica_groups=[list(range(NCORES))],
                ins=[zin[:].opt()],
                outs=[zout[:].opt()],
            )
            # zq_all[qq, n, r] = zflat[2048*r + 128*n + qq] = zrow[r, 128n+qq]
            for r in range(4):
                nc.gpsimd.dma_start(
                    zq_all[:, :, r],
                    zout[r * D : (r + 1) * D].rearrange("(n q) -> q n", q=P),
                )



            # ---- middle groups: GEMM, bank-freeing copies, deferred
            # finishing of the previous group's chunks ----
            for g in range(1, len(GROUPS) - 1):
                pss = group_psums(g)
                vts = {}
                prev = list(pending)
                pending.clear()
                for idx, kk in enumerate(KORDER):
                    group_matmuls(g, pss, kk, idx, vts)
                for ci, n in enumerate(GROUPS[g]):
                    copy_chunk(n, pss[ci * NMB], pss[ci * NMB + 1])
                just = list(pending)
                pending.clear()
                pending.extend(prev)
                finish_pending()
                pending.extend(just)

            # ---- final group: two 2-chunk pairs on disjoint banks so
            # each pair's epilogue overlaps the next pair's GEMM ----
            PAIR_TAGS = [["ps0", "ps1", "ps2", "ps3"],
                         ["ps4", "ps5", "pz0", "pz1"]]
            for pi, pair in enumerate(([12, 13], [14, 15])):
                pps = [
                    psg.tile([P, MBS], f32, name=f"pp{pi}_{s}",
                             tag=PAIR_TAGS[pi][s])
                    for s in range(4)
                ]
                for idx, kk in enumerate(KORDER):
                    vt = vp.tile([P, 2 * P], f32r, name="vt2", tag="vt2",
                                  bufs=6)
                    nc.sync.dma_start(
                        vt[:],
                        v[kk * P : (kk + 1) * P,
                          pair[0] * P : (pair[1] + 1) * P],
                    )
                    for ci in range(2):
                        for mb in range(NMB):
                            nc.tensor.matmul(
                                pps[ci * NMB + mb][:],
                                vt[:, ci * P : (ci + 1) * P],
                                xts[kk][:, mb * MBS : (mb + 1) * MBS],
                                start=(idx == 0), stop=(idx == KT - 1),
                            )
                for ci, n in enumerate(pair):
                    copy_chunk(n, pps[ci * NMB], pps[ci * NMB + 1])
                finish_pending()

    nc.compile()
    return nc


def _get_nc():
    nc = _cache.get("nc")
    if nc is None:
        nc = _build()
        _cache["nc"] = nc
    return nc


def kernel(e1, e2, W, V, b):
    from concourse.bass_utils import run_bass_kernel_spmd

    nc = _get_nc()

    e1 = np.ascontiguousarray(np.asarray(e1, dtype=np.float32))
    e2 = np.ascontiguousarray(np.asarray(e2, dtype=np.float32))
    W = np.ascontiguousarray(np.asarray(W, dtype=np.float32))
    V = np.ascontiguousarray(np.asarray(V, dtype=np.float32))
    b = np.ascontiguousarray(np.asarray(b, dtype=np.float32))

    in_maps = []
    for i in range(NCORES):
        sl = slice(i * M, (i + 1) * M)
        xt_i = np.ascontiguousarray(
            np.concatenate([e1[sl].T, e2[sl].T], axis=0)
        )
        qlo = (i % 2) * M
        # 1/K scale folded into W (power of two — exact in fp32);
        # bf16 is plenty for the z term (|z| ~ 1e-3 vs |ff| ~ 1)
        wt_i = np.ascontiguousarray(
            (W[qlo : qlo + M].T * np.float32(1.0 / K)).astype(
                ml_dtypes.bfloat16
            )
        )
        in_maps.append({"xt": xt_i, "v": V, "wt": wt_i, "bvec": b})

    res = run_bass_kernel_spmd(nc, in_maps, list(range(NCORES)))
    global LAST_RESULT
    LAST_RESULT = res

    out = np.empty((B, K), dtype=np.float32)
    for i in range(NCORES):
        out[i * M : (i + 1) * M, :] = res.results[i]["out_t"].T
    return out
